# revision 11
# baseline (speedup 1.0000x reference)
"""Trainium2 Bass kernel: causal multi-head attention (B=4, T=2048, D=1024, H=16).

Sharding: tensor-parallel over heads. Each of the 8 cores handles 2 heads
(a 128-wide slice of the head dimension): it computes q/k/v projections for
its heads, causal attention, and a partial output projection
y_partial = o_local @ wo_local^T.  The full output is the sum of the 8
partials (reduced on host).

All matmul data is bf16 (fp32 accumulation in PSUM); rel-err budget is 2e-2
and bf16 lands ~1e-3.

Device dataflow per core:
  qT,kT = w_loc @ x^T          [128, T] bf16   (PSUM accum over 8 k-subtiles)
  v     = x @ wv_loc^T         computed directly in natural [token, hd]
                               layout (stationary = x^T chunk, moving = wv)
                               so no PE transpose is needed; stored per
                               128-token block in va, augmented with a ones
                               column so PV also produces row sums Z.
  S^T block = kT_blk^T-contract qT    [tk=128, tq<=512]; the two heads'
                               matmuls sit in row groups 0-63 / 64-127 of
                               the PE array and execute concurrently.
  E = exp(S^T * scale)         (ACT), causal keep-mask on diagonal blocks.
  PV: po[65, h, tq] += [v|1]^T-contract E  -> rows 0:64 = o_un^T, row 64 = Z
  oT = po rows, normalized by 1/Z (reciprocal_approx_fast + gpsimd
                               partition-broadcast), per query tile.
  y_chunk = oT_chunk^T @ wo^T  [tq=128, 512] -> DMA to DRAM, per query tile
                               (pipelined into the attention of later tiles).

Emission interleaves next-batch projections and current-batch output
projection between attention query tiles so the PE always has dense fill
work (keeps the HAM clock gate at 2.4 GHz) while ACT works through the exps.
"""

import numpy as np

import concourse.bass as bass
import concourse.bacc as bacc
import concourse.mybir as mybir
from concourse.tile import TileContext
from contextlib import ExitStack

# Problem constants (hardcoded per contract)
B, T, D, H = 4, 2048, 1024, 16
HD = D // H            # 64 head dim
P = 128                # partitions
KO = D // P            # 8 contraction subtiles for projections
TQT = 512              # tq tile width
NBLK = T // P          # 16 tk blocks per batch
NHL = 2                # heads per core
TT = B * T             # 8192 tokens
NCORES = 8
SCALE = 1.0 / float(np.sqrt(np.float32(HD)))

F32 = mybir.dt.float32
BF16 = mybir.dt.bfloat16


def build_program():
    nc = bacc.Bacc("TRN2", target_bir_lowering=False, num_devices=NCORES)
    xt = nc.dram_tensor("xt", [D, TT], BF16, kind="ExternalInput")
    wq = nc.dram_tensor("wq", [D, P], BF16, kind="ExternalInput")
    wk = nc.dram_tensor("wk", [D, P], BF16, kind="ExternalInput")
    wv = nc.dram_tensor("wv", [D, P], BF16, kind="ExternalInput")
    wo = nc.dram_tensor("wo", [P, D], BF16, kind="ExternalInput")
    cm = nc.dram_tensor("cmask", [4, P, TQT], BF16, kind="ExternalInput")
    y = nc.dram_tensor("y", [TT, D], F32, kind="ExternalOutput")

    xt_r = xt[:].rearrange("(ko p) t -> p ko t", p=P)
    y_r = y[:].rearrange("(tn p) c -> p tn c", p=P)

    Exp = mybir.ActivationFunctionType.Exp
    Mult = mybir.AluOpType.mult

    with TileContext(nc) as tc, ExitStack() as ctx:
        const = ctx.enter_context(tc.tile_pool(name="const", bufs=1))
        qk_pool = ctx.enter_context(tc.tile_pool(name="qk", bufs=2))
        va_pool = ctx.enter_context(tc.tile_pool(name="va", bufs=2))
        o_pool = ctx.enter_context(tc.tile_pool(name="o", bufs=2))
        xt_pool = ctx.enter_context(tc.tile_pool(name="xtp", bufs=3))
        e_pool = ctx.enter_context(tc.tile_pool(name="e", bufs=3))
        z_pool = ctx.enter_context(tc.tile_pool(name="z", bufs=2))
        y_pool = ctx.enter_context(tc.tile_pool(name="yp", bufs=4))
        psS = ctx.enter_context(tc.tile_pool(name="psS", bufs=2, space="PSUM"))
        psO = ctx.enter_context(tc.tile_pool(name="psO", bufs=1, space="PSUM"))
        psA = ctx.enter_context(tc.tile_pool(name="psA", bufs=2, space="PSUM"))

        # --- constants into SBUF ---
        # wq + first x tile gate the first matmul; cmask/wo are not needed
        # until much later, so they go last in the DMA queue.
        wq_sb = const.tile([P, KO, P], BF16, tag="wq")
        wk_sb = const.tile([P, KO, P], BF16, tag="wk")
        wv_sb = const.tile([P, KO, P], BF16, tag="wv")
        wo_sb = const.tile([P, D], BF16, tag="wo")
        cm_sb = const.tile([P, 4, TQT], BF16, tag="cm")
        for w_sb, w_d in ((wq_sb, wq), (wk_sb, wk), (wv_sb, wv)):
            nc.sync.dma_start(w_sb, w_d[:].rearrange("(ko p) d -> p ko d", p=P))

        def a_alloc(b):
            """Allocate batch b's tile set and init the ones column."""
            qT = qk_pool.tile([P, T], BF16, tag="qT", name=f"qT{b}")
            kT = qk_pool.tile([P, T], BF16, tag="kT", name=f"kT{b}")
            va = va_pool.tile([P, NBLK, NHL, HD + 1], BF16, tag="va", name=f"va{b}")
            oT = o_pool.tile([P, T], BF16, tag="oT", name=f"oT{b}")
            nc.vector.tensor_copy(
                va[:, :, :, HD : HD + 1],
                nc.const_aps.tensor(1.0, (P, NBLK, NHL, 1), BF16),
            )
            return qT, kT, va, oT

        def a_tile(b, tiles, tt):
            """q/k/v projections for one 512-token tile of batch b.

            q,k: stationary weight chunks, output in transposed [hd, token]
            layout.
            v: stationary x^T chunk per 128-token block, moving wv, output in
            natural [token, hd] layout (both heads side by side)."""
            qT, kT, va, oT = tiles
            xx = xt_pool.tile([P, KO, TQT], BF16, tag="xt")
            nc.sync.dma_start(
                xx, xt_r[:, :, b * T + tt * TQT : b * T + (tt + 1) * TQT]
            )
            for w_sb, dst in ((wq_sb, qT), (wk_sb, kT)):
                pps = psA.tile([P, TQT], F32, tag="psA", name="pps")
                for ko in range(KO):
                    nc.tensor.matmul(
                        pps,
                        w_sb[:, ko, :],
                        xx[:, ko, :],
                        start=(ko == 0),
                        stop=(ko == KO - 1),
                    )
                nc.vector.tensor_copy(dst[:, tt * TQT : (tt + 1) * TQT], pps)
            for j in range(4):
                psv = psA.tile([P, P], F32, tag="psA", name="psv")
                for ko in range(KO):
                    nc.tensor.matmul(
                        psv,
                        xx[:, ko, j * P : (j + 1) * P],
                        wv_sb[:, ko, :],
                        start=(ko == 0),
                        stop=(ko == KO - 1),
                    )
                blk = tt * 4 + j
                nc.vector.tensor_copy(
                    va[:, blk, :, 0:HD],
                    psv[:].rearrange("p (h d) -> p h d", h=NHL),
                )

        def b_qt(b, tiles, qt):
            """Attention for query tile qt of batch b: S -> exp -> PV."""
            qT, kT, va, oT = tiles
            tq0 = qt * TQT
            nblk = tq0 // P + TQT // P
            po = psO.tile([HD + 1, NHL, TQT], F32, tag="po")
            for kb in range(nblk):
                m = kb - tq0 // P  # >=0: diagonal-crossing block
                c0 = P * m if m >= 0 else 0
                ps2 = psS.tile([P, NHL, TQT], F32, tag="ps")
                for h in range(NHL):
                    hs = slice(h * HD, (h + 1) * HD)
                    nc.tensor.matmul(
                        ps2[:, h, c0:TQT],
                        kT[hs, kb * P : (kb + 1) * P],
                        qT[hs, tq0 + c0 : tq0 + TQT],
                        start=True,
                        stop=True,
                    )
                et = e_pool.tile([P, NHL, TQT], BF16, tag="et")
                nc.scalar.activation(
                    et[:, :, c0:TQT], ps2[:, :, c0:TQT], Exp, scale=SCALE
                )
                if m >= 0:
                    nc.vector.tensor_tensor(
                        et[:, :, c0 : c0 + P],
                        et[:, :, c0 : c0 + P],
                        cm_sb[:, m : m + 1, c0 : c0 + P].to_broadcast(
                            (P, NHL, P)
                        ),
                        Mult,
                    )
                for h in range(NHL):
                    nc.tensor.matmul(
                        po[:, h, c0:TQT],
                        va[:, kb, h, :],
                        et[:, h, c0:TQT],
                        start=(kb == 0),
                        stop=(kb == nblk - 1),
                    )
            return po

        def tail_qt(b, tiles, qt, po):
            """Evacuate po: oT rows, then 1/Z normalize the 512-token slice."""
            qT, kT, va, oT = tiles
            tq0 = qt * TQT
            zq = z_pool.tile([P, TQT], F32, tag="zq")
            nc.gpsimd.memset(zq, 1.0)
            for h in range(NHL):
                hs = slice(h * HD, (h + 1) * HD)
                nc.vector.tensor_copy(oT[hs, tq0 : tq0 + TQT], po[0:HD, h, :])
                nc.vector.tensor_copy(
                    zq[h * HD : h * HD + 1, :], po[HD : HD + 1, h, :]
                )
            rc = z_pool.tile([P, TQT], F32, tag="rc")
            nc.vector.reciprocal_approx_fast(rc, zq)
            rcb = z_pool.tile([P, TQT], BF16, tag="rcb")
            nc.vector.tensor_copy(rcb, rc)
            for h in range(NHL):
                hs = slice(h * HD, (h + 1) * HD)
                rr = z_pool.tile([1, TQT], BF16, tag="rr", name=f"rr{h}")
                nc.vector.tensor_copy(rr, rcb[h * HD : h * HD + 1, :])
                rzb = z_pool.tile([P, TQT], BF16, tag="rzb", name=f"rzb{h}")
                nc.gpsimd.partition_broadcast(rzb, rr)
                nc.vector.tensor_tensor(
                    oT[hs, tq0 : tq0 + TQT],
                    oT[hs, tq0 : tq0 + TQT],
                    rzb[hs, :],
                    Mult,
                )

        def c_qt(b, tiles, qt):
            """Partial output projection for query tile qt of batch b."""
            qT, kT, va, oT = tiles
            for tn in range(qt * 4, (qt + 1) * 4):
                for cc in range(D // TQT):
                    psy = psA.tile([P, TQT], F32, tag="psA")
                    nc.tensor.matmul(
                        psy,
                        oT[:, tn * P : (tn + 1) * P],
                        wo_sb[:, cc * TQT : (cc + 1) * TQT],
                        start=True,
                        stop=True,
                    )
                    yt = y_pool.tile([P, TQT], F32, tag="yt")
                    nc.any.tensor_copy(yt, psy)
                    nc.sync.dma_start(
                        y_r[:, b * (T // P) + tn, cc * TQT : (cc + 1) * TQT], yt
                    )

        # Software pipeline: next batch's projections and this batch's output
        # projection are emitted between query tiles so the PE has fill work
        # during the exp round trips. cmask/wo DMAs go after the first x tile
        # so they don't delay the first projection matmuls.
        tiles = {0: a_alloc(0)}
        a_tile(0, tiles[0], 0)
        nc.sync.dma_start(wo_sb, wo[:])
        nc.sync.dma_start(cm_sb, cm[:].rearrange("m p t -> p m t"))
        for tt in range(1, 4):
            a_tile(0, tiles[0], tt)
        for b in range(B):
            if b + 1 < B:
                tiles[b + 1] = a_alloc(b + 1)
            for qt in range(T // TQT):
                po = b_qt(b, tiles[b], qt)
                tail_qt(b, tiles[b], qt, po)
                c_qt(b, tiles[b], qt)
                if b + 1 < B:
                    a_tile(b + 1, tiles[b + 1], qt)
            del tiles[b]

    nc.compile()
    return nc


def make_core_inputs(x, wq, wk, wv, wo):
    """Host-side sharding/layout prep. Returns list of 8 in_maps."""
    import ml_dtypes

    bf16 = ml_dtypes.bfloat16
    x = np.asarray(x, dtype=np.float32)
    wq = np.asarray(wq, dtype=np.float32).astype(bf16)
    wk = np.asarray(wk, dtype=np.float32).astype(bf16)
    wv = np.asarray(wv, dtype=np.float32).astype(bf16)
    wo = np.asarray(wo, dtype=np.float32).astype(bf16)

    xt = np.ascontiguousarray(x.reshape(TT, D).T).astype(bf16)  # [D, TT]
    # causal keep-masks for diagonal-crossing blocks, 4 shift variants
    i = np.arange(P)[:, None]
    j = np.arange(TQT)[None, :]
    cmask = np.stack(
        [(i + P * m <= j).astype(bf16) for m in range(4)], axis=0
    )  # [4, P, TQT]

    in_maps = []
    for c in range(NCORES):
        dr = slice(c * P, (c + 1) * P)
        in_maps.append(
            {
                "xt": xt,
                "wq": np.ascontiguousarray(wq[dr, :].T),
                "wk": np.ascontiguousarray(wk[dr, :].T),
                "wv": np.ascontiguousarray(wv[dr, :].T),
                "wo": np.ascontiguousarray(wo[:, dr].T),
                "cmask": cmask,
            }
        )
    return in_maps


_CACHE = {}


def run(in_maps, **kwargs):
    from concourse.bass_utils import run_bass_kernel_spmd

    if "nc" not in _CACHE:
        _CACHE["nc"] = build_program()
    nc = _CACHE["nc"]
    res = run_bass_kernel_spmd(nc, in_maps, core_ids=list(range(NCORES)), **kwargs)
    return res


def kernel(x, wq, wk, wv, wo):
    in_maps = make_core_inputs(x, wq, wk, wv, wo)
    res = run(in_maps)
    y = np.zeros((TT, D), dtype=np.float32)
    for r in res.results:
        y += r["y"]
    return y.reshape(B, T, D)
